# revision 11
# baseline (speedup 1.0000x reference)
"""Causal self-attention (B=4, T=2048, C=768, H=6, D=128) on 8 trn2 NeuronCores.

Sharding: 24 (batch, head) units -> 8 cores, each core owns 1 batch x 3 heads.
Unshard: out[b] = partial[core 2b] + partial[core 2b+1]  (tensor-parallel sum).

v5 design notes (vs v4 two-phase):
  - INTERLEAVED schedule with a 9-tile stage-1 prefix: v4's strict two-phase
    ran 145us of gpsimd/DVE-bound stage-1 with the PE at 35%, then 137us of
    PE-bound attention with the vector engines idle. v5 front-loads 9 tiles
    of stage-1 (vector-bound, PE warms up on QKV), then spreads the 7
    remaining stage-1 tiles + transposes into attention chunks 0-2 (whose PE
    work they hide under); chunk 3 (the PE-fattest) runs pure.
  - Chunk qc's first score matmul reads QT columns for q-tiles 4qc..4qc+3,
    so those transposes must complete BEFORE the chunk starts (not
    just-in-time per key tile) — the extras tables below respect that.
  - stage-1 op fixes: rope swap-halves via strided 2x-mode DVE TT reading
    qk_sb directly (v4's materialized swap cost 2.9us/tile of gpsimd);
    cos/sin host-expanded to [T,768] with signs baked so every operand is
    contiguous; engine split tuned to DVE 4.4us / gpsimd 4.6us per tile.
  - proj of chunk qc-1 is inserted after head 1 (not head 0) of chunk qc so
    the chunk_end reciprocal/broadcast chain can finish without stalling
    the PE FIFO.
  - softmax denominator reciprocal via reciprocal_approx_fast (fp32, ~18
    bits); output in bf16 (halves out DMA); host sums TP pairs in fp32.
"""

import numpy as np
import ml_dtypes

import concourse.bacc as bacc
import concourse.bass as bass
import concourse.mybir as mybir
from concourse import tile
from concourse.bass_utils import run_bass_kernel_spmd

F32 = mybir.dt.float32
BF16 = mybir.dt.bfloat16
AF = mybir.ActivationFunctionType
ALU = mybir.AluOpType

B, T, C, H, D = 4, 2048, 768, 6, 128
HALF = D // 2
NH = 3            # heads per core
CT = C // 128     # 6 contraction tiles for projections
NT = T // 128     # 16 token tiles
QC = 512          # query-chunk width for attention
NQC = T // QC     # 4 chunks
SCALE = 1.0 / float(np.sqrt(D))
EPS = 1e-6
PREFIX = 9        # stage-1 tiles emitted before attention starts
USE_POW = False   # pow ALU op fails neuronx-cc compile

_CACHE = {}


def _build_nc():
    nc = bacc.Bacc("TRN2")

    xT = nc.dram_tensor("xT", [C, T], BF16, kind="ExternalInput")
    wqT = nc.dram_tensor("wqT", [C, NH * D], BF16, kind="ExternalInput")
    wkT = nc.dram_tensor("wkT", [C, NH * D], BF16, kind="ExternalInput")
    wvT = nc.dram_tensor("wvT", [C, NH * D], BF16, kind="ExternalInput")
    wpT = nc.dram_tensor("wpT", [NH * D, C], BF16, kind="ExternalInput")
    cosr = nc.dram_tensor("cosr", [T, 2 * NH * D], BF16, kind="ExternalInput")
    sinr = nc.dram_tensor("sinr", [T, 2 * NH * D], BF16, kind="ExternalInput")
    maskC = nc.dram_tensor("maskC", [128, QC], BF16, kind="ExternalInput")
    ident = nc.dram_tensor("ident", [128, 128], BF16, kind="ExternalInput")
    ones_in = nc.dram_tensor("ones_in", [128, 1], BF16, kind="ExternalInput")
    out = nc.dram_tensor("out", [T, C], BF16, kind="ExternalOutput")

    with tile.TileContext(nc) as tc:
        with (
            tc.tile_pool(name="persist", bufs=1) as persist,
            tc.tile_pool(name="qkvbuf", bufs=1) as qkvbuf,
            tc.tile_pool(name="wbuf", bufs=1) as wbuf,
            tc.tile_pool(name="qkp", bufs=3) as qkpool,
            tc.tile_pool(name="rope", bufs=4) as rpool,
            tc.tile_pool(name="nrmp", bufs=10) as npool,
            tc.tile_pool(name="stat", bufs=6) as spool,
            tc.tile_pool(name="att", bufs=5) as apool,
            tc.tile_pool(name="acc", bufs=2) as accpool,
            tc.tile_pool(name="ybuf", bufs=2) as ypool,
            tc.tile_pool(name="obuf", bufs=3) as opool,
        ):
            QT = qkvbuf.tile([128, NH, T], BF16)       # [d, h, t]
            KT = qkvbuf.tile([128, NH, T], BF16)       # [d, h, t]
            V = qkvbuf.tile([128, NT, NH * D], BF16)   # [s%128, s//128, h*D+d]
            ones = persist.tile([128, 1], BF16)
            ones_row = persist.tile([1, 128], BF16)
            idn = persist.tile([128, 128], BF16)
            mask = persist.tile([128, QC], BF16)
            wp_sb = persist.tile([128, NH, C], BF16)   # [d, h, c]

            wq_sb = wbuf.tile([128, CT, NH * D], BF16)
            wk_sb = wbuf.tile([128, CT, NH * D], BF16)
            wv_sb = wbuf.tile([128, CT, NH * D], BF16)
            x_sb = wbuf.tile([128, CT, T], BF16)       # [c%128, c//128, t]

            wqT_r = wqT.rearrange("(ci p) o -> p ci o", p=128)
            nc.sync.dma_start(wq_sb[:], wqT_r[:])
            xT_r = xT.rearrange("(ci p) (g t) -> p ci g t", p=128, g=8)
            x_sb_g = x_sb[:].rearrange("p ci (g t) -> p ci g t", g=8)
            nc.sync.dma_start(x_sb_g[:, :, 0], xT_r[:, :, 0])
            nc.sync.dma_start(wk_sb[:], wkT.rearrange("(ci p) o -> p ci o", p=128))
            nc.sync.dma_start(wv_sb[:], wvT.rearrange("(ci p) o -> p ci o", p=128))
            for g in range(1, 8):
                nc.sync.dma_start(x_sb_g[:, :, g], xT_r[:, :, g])

            # [c|c] per (m, h) and [s|-s] per (m, h), host-expanded
            cos_sb = wbuf.tile([128, NT, 2 * NH * D], BF16)
            sin_sb = wbuf.tile([128, NT, 2, NH, 2, HALF], BF16)
            nc.sync.dma_start(cos_sb[:], cosr.rearrange("(tt p) f -> p tt f", p=128))
            nc.sync.dma_start(
                sin_sb[:].rearrange("p tt m h two f -> p tt (m h two f)"),
                sinr.rearrange("(tt p) f -> p tt f", p=128))
            nc.sync.dma_start(idn[:], ident[:])
            nc.sync.dma_start(mask[:], maskC[:])
            nc.sync.dma_start(ones[:], ones_in[:])
            nc.sync.dma_start(wp_sb[:], wpT.rearrange("(h p) c -> p h c", p=128))

            nc.vector.memset(ones_row[:], 1.0)

            out_r = out.rearrange("(tt p) c -> p tt c", p=128)

            # ---------------- stage 1 emitters ------------------------------
            nrm_of = {}
            s1_state = {}
            pools = {}

            def stage1_a(tt):
                # q in bank 0, k in bank 1, v in bank 2 of one psum tile
                qkv = pools["psq"].tile([128, 3, 512], F32, tag="qkv")
                for ci in range(CT):
                    st_, sp_ = (ci == 0), (ci == CT - 1)
                    lhs = x_sb[:, ci, tt * 128:(tt + 1) * 128]
                    nc.tensor.matmul(qkv[:, 0, 0:NH * D], lhs, wq_sb[:, ci, :],
                                     start=st_, stop=sp_, skip_group_check=True)
                    nc.tensor.matmul(qkv[:, 1, 0:NH * D], lhs, wk_sb[:, ci, :],
                                     start=st_, stop=sp_, skip_group_check=True)
                    nc.tensor.matmul(qkv[:, 2, 0:NH * D], lhs, wv_sb[:, ci, :],
                                     start=st_, stop=sp_, skip_group_check=True)

                # V: straight copy PSUM -> SBUF (bf16) in natural [t, o]
                nc.scalar.copy(V[:, tt, :], qkv[:, 2, 0:NH * D])

                # park q,k in SBUF bf16 (ACT reads PSUM)
                qk_sb = qkpool.tile([128, 2, NH, D], BF16, tag="qksb")
                nc.scalar.copy(qk_sb[:].rearrange("p m h f -> p m (h f)"),
                               qkv[:, 0:2, 0:NH * D])

                # rope: r = qk (.) [c|c]  +  swap(qk) (.) [s|-s]
                # all big TT on DVE (2x mode); w halves as strided 2x TT
                # reading qk_sb directly (no materialized swap).
                u = rpool.tile([128, 2, NH, D], BF16, tag="u")
                w = rpool.tile([128, 2, NH, D], BF16, tag="w")
                r = rpool.tile([128, 2, NH, D], BF16, tag="r")
                nc.vector.tensor_mul(u[:].rearrange("p m h f -> p (m h f)"),
                                     qk_sb[:].rearrange("p m h f -> p (m h f)"),
                                     cos_sb[:, tt])
                nc.vector.tensor_mul(w[:, :, :, 0:HALF], qk_sb[:, :, :, HALF:D],
                                     sin_sb[:, tt, :, :, 0, :])
                nc.vector.tensor_mul(w[:, :, :, HALF:D], qk_sb[:, :, :, 0:HALF],
                                     sin_sb[:, tt, :, :, 1, :])
                nc.vector.tensor_add(r[:], u[:], w[:])

                r6 = r[:].rearrange("p m h f -> p (m h) f")
                sums = spool.tile([128, 2 * NH], F32, tag="sums")
                nc.vector.tensor_reduce(sums[:], r6, axis=mybir.AxisListType.X,
                                        op=ALU.add)
                junk = rpool.tile([128, 2, NH, D], BF16, tag="junk")
                nc.vector.tensor_mul(junk[:], r[:], r[:])
                sumsq = spool.tile([128, 2 * NH], F32, tag="sumsq")
                nc.vector.tensor_reduce(
                    sumsq[:], junk[:].rearrange("p m h f -> p (m h) f"),
                    axis=mybir.AxisListType.X, op=ALU.add)
                negmean = spool.tile([128, 2 * NH], F32, tag="negmean")
                nc.vector.tensor_scalar_mul(negmean[:], sums[:], -1.0 / D)
                var = spool.tile([128, 2 * NH], F32, tag="var")
                nc.vector.tensor_mul(var[:], negmean[:], sums[:])
                nc.vector.tensor_add(var[:], sumsq[:], var[:])
                s1_state[tt] = (r, negmean, var)

            def stage1_b(tt):
                r, negmean, var = s1_state.pop(tt)
                rstd = spool.tile([128, 2 * NH], F32, tag="rstd")
                if USE_POW:
                    # rstd = (var/(D-1))^-0.5; the +eps on std is a ~2e-6
                    # relative effect here and is dropped
                    nc.vector.tensor_scalar(rstd[:], var[:], 1.0 / (D - 1), -0.5,
                                            op0=ALU.mult, op1=ALU.pow)
                else:
                    stdv = spool.tile([128, 2 * NH], F32, tag="stdv")
                    nc.scalar.activation(stdv[:], var[:], AF.Sqrt,
                                         scale=1.0 / (D - 1))
                    nc.vector.tensor_scalar_add(stdv[:], stdv[:], EPS)
                    nc.vector.reciprocal(rstd[:], stdv[:])

                # normalize: nrm = (r + negmean) * rstd  (q on DVE, k on gpsimd)
                nrm = npool.tile([128, 2, NH, D], BF16, tag="nrm")
                for mi in range(2):
                    for h in range(NH):
                        c = mi * NH + h
                        nc.vector.tensor_scalar(
                            nrm[:, mi, h], r[:, mi, h],
                            negmean[:, c:c + 1], rstd[:, c:c + 1],
                            op0=ALU.add, op1=ALU.mult)
                nrm_of[tt] = nrm

            def stage1_emit(tt):
                stage1_a(tt)
                stage1_b(tt)

            def emit_tp(tt):
                # SBUF->SBUF transpose via the DMA XBAR path: costs no PE,
                # ACT or PSUM; latency hidden by the schedule slack
                nrm = nrm_of.pop(tt)
                for mi in range(2):
                    dstT = QT if mi == 0 else KT
                    for h in range(NH):
                        nc.sync.dma_start_transpose(
                            dstT[:, h, tt * 128:(tt + 1) * 128], nrm[:, mi, h])

            # ---------------- attention chunk emitters -----------------------
            def emit_proj(qc, yTc):
                quanta = []
                for j in range(QC // 128):
                    def thunk(j=j, qc=qc, yTc=yTc):
                        tt = qc * (QC // 128) + j
                        op0 = pools["psS"].tile([128, C // 2], F32, tag="ps")
                        op1 = pools["psS"].tile([128, C // 2], F32, tag="ps")
                        for h in range(NH):
                            lhs = yTc[:, h, j * 128:(j + 1) * 128]
                            nc.tensor.matmul(op0[:], lhs, wp_sb[:, h, 0:C // 2],
                                             start=(h == 0), stop=(h == NH - 1))
                            nc.tensor.matmul(op1[:], lhs, wp_sb[:, h, C // 2:C],
                                             start=(h == 0), stop=(h == NH - 1))
                        ot = opool.tile([128, C], BF16, tag="ot")
                        nc.scalar.copy(ot[:, 0:C // 2], op0[:])
                        nc.scalar.copy(ot[:, C // 2:C], op1[:])
                        nc.sync.dma_start(out_r[:, tt, :], ot[:])
                    quanta.append(thunk)
                return quanta

            def attn_quanta(qc, pending_proj):
                Q0 = qc * QC
                n_st = (Q0 + QC) // 128
                state = {}
                ets = {}

                def start_chunk():
                    state["dps"] = pools["psD"].tile([128, QC], F32, tag="dps", name="dps")
                    state["yU"] = ypool.tile([128, NH, QC], BF16, tag="yU",
                                             name="yU")

                def loc0_of(st):
                    j = st - (n_st - 4)
                    return 128 * j if j > 0 else 0

                def emit_score(h, st):
                    loc0 = loc0_of(st)
                    diag = st * 128 >= Q0
                    sps = pools["psS"].tile([128, QC], F32, tag="ps")
                    nc.tensor.matmul(
                        sps[:, loc0:QC],
                        KT[:, h, st * 128:(st + 1) * 128],
                        QT[:, h, Q0 + loc0:Q0 + QC],
                        start=True, stop=not diag)
                    if diag:
                        # additive causal mask (0 / -1e30) folded into the
                        # scores psum via one identity-lhsT matmul: keeps the
                        # score->exp->AV chain off the vector engines
                        nc.tensor.matmul(
                            sps[:, loc0:QC], idn[:], mask[:, 0:QC - loc0],
                            start=False, stop=True)
                    et = apool.tile([128, QC], BF16, tag="et")
                    nc.scalar.activation(et[:, loc0:QC], sps[:, loc0:QC],
                                         AF.Exp, scale=SCALE)
                    ets[(h, st)] = et

                def emit_av(h, st):
                    loc0 = loc0_of(st)
                    et = ets.pop((h, st))
                    nc.tensor.matmul(
                        state["yps"][:, loc0:QC],
                        V[:, st, h * D:(h + 1) * D],
                        et[:, loc0:QC],
                        start=(st == 0), stop=(st == n_st - 1),
                        skip_group_check=True)
                    nc.tensor.matmul(
                        state["dps"][32 * h:32 * h + 1, loc0:QC],
                        ones[:],
                        et[:, loc0:QC],
                        start=(st == 0), stop=(st == n_st - 1),
                        skip_group_check=True)

                quanta = [start_chunk]
                for h in range(NH):
                    def head_start(h=h):
                        state["yps"] = pools["psY"].tile([128, QC], F32, tag="yps",
                                                name="yps")
                        emit_score(h, 0)
                        if n_st > 1:
                            emit_score(h, 1)
                    quanta.append(head_start)
                    for st in range(n_st):
                        def block(h=h, st=st):
                            if st + 2 < n_st:
                                emit_score(h, st + 2)
                            emit_av(h, st)
                        quanta.append(block)
                    if h == 1 and pending_proj:
                        quanta.extend(pending_proj)

                    def head_end(h=h):
                        nc.scalar.copy(state["yU"][:, h, :], state["yps"][:])
                    quanta.append(head_end)

                def chunk_end():
                    dps, yU = state["dps"], state["yU"]
                    rcf = accpool.tile([65, QC], F32, tag="rcf")
                    nc.vector.reciprocal_approx_fast(rcf[:], dps[0:65, :])
                    yTc = ypool.tile([128, NH, QC], BF16, tag="yT")
                    for h in range(NH):
                        # stage row at partition 0, then broadcast across
                        # partitions with a K=1 matmul (ones_row (x) rch) --
                        # much lower latency than gpsimd partition_broadcast
                        rch = accpool.tile([1, QC], BF16, tag="rch")
                        nc.vector.tensor_copy(rch[:], rcf[32 * h:32 * h + 1, :])
                        rbc = pools["psS"].tile([128, QC], F32, tag="ps")
                        nc.tensor.matmul(rbc[:], ones_row[:], rch[:],
                                         start=True, stop=True)
                        nc.vector.tensor_mul(yTc[:, h, :], yU[:, h, :], rbc[:])
                    state["yTc"] = yTc
                quanta.append(chunk_end)
                return quanta, state

            # ---------------- emission schedule ------------------------------
            # Prefix runs with double-buffered qkv psum (6 banks) + psT x2 —
            # the attention banks aren't needed yet. The scopes then swap.
            with (
                tc.tile_pool(name="psQKVa", bufs=2, space="PSUM") as psq_a,
            ):
                pools["psq"] = psq_a
                for tt in range(PREFIX):
                    stage1_emit(tt)
                for tt in range(4):
                    emit_tp(tt)

            def sa(tt):
                return lambda: stage1_a(tt)

            def sb(*tts):
                # paired: keeps the Sqrt ACT ops adjacent (one table reload
                # per pair instead of one per tile when interleaved with Exp)
                def thunk():
                    for tt in tts:
                        stage1_b(tt)
                return thunk

            def t(tt):
                return lambda: emit_tp(tt)

            extras = {
                0: [t(4), sa(9), t(5), sa(10), sb(9, 10), t(6), t(7)],
                1: [sa(11), t(8), sa(12), sb(11, 12), t(9), t(10), t(11)],
                2: [sa(13), sa(14), sb(13, 14), t(12), sa(15), sb(15), t(13),
                    t(14), t(15)],
                3: [],
            }

            with (
                tc.tile_pool(name="psQKVb", bufs=1, space="PSUM") as psq_b,
                tc.tile_pool(name="psS", bufs=2, space="PSUM") as psS_p,
                tc.tile_pool(name="psY", bufs=2, space="PSUM") as psY_p,
                tc.tile_pool(name="psD", bufs=1, space="PSUM") as psD_p,
            ):
                pools["psq"] = psq_b
                pools["psS"], pools["psY"], pools["psD"] = psS_p, psY_p, psD_p
                pending_proj = None
                for qc in range(NQC):
                    quanta, state = attn_quanta(qc, pending_proj)
                    ext = extras[qc]
                    stride = max(1, len(quanta) // (len(ext) + 1)) if ext else 0
                    ei = 0
                    for i, q in enumerate(quanta):
                        q()
                        if ext and ei < len(ext) and i % stride == stride - 1:
                            ext[ei]()
                            ei += 1
                    while ext and ei < len(ext):
                        ext[ei]()
                        ei += 1
                    pending_proj = emit_proj(qc, state["yTc"])
                for q in pending_proj:
                    q()

    nc.compile()
    return nc


def _get_nc():
    if "nc" not in _CACHE:
        _CACHE["nc"] = _build_nc()
    return _CACHE["nc"]


def _in_maps(x, cos, sin, wq, wk, wv, wproj):
    bf = ml_dtypes.bfloat16
    cos_np = np.asarray(cos, dtype=np.float32)
    sin_np = np.asarray(sin, dtype=np.float32)
    # cosr: [c|c] per (m, h); sinr: [s|-s] per (m, h); both [T, 768]
    cosr = np.ascontiguousarray(
        np.tile(np.concatenate([cos_np, cos_np], axis=1), (1, 2 * NH)).astype(bf))
    sinr = np.ascontiguousarray(
        np.tile(np.concatenate([sin_np, -sin_np], axis=1), (1, 2 * NH)).astype(bf))
    maskC = np.ascontiguousarray(
        np.where(np.arange(QC)[None, :] >= np.arange(128)[:, None],
                 0.0, -1e30).astype(bf))
    ident = np.eye(128, dtype=bf)
    maps = []
    for c in range(8):
        b = c // 2
        hs = (c % 2) * NH
        sl = slice(hs * D, (hs + NH) * D)
        maps.append({
            "xT": np.ascontiguousarray(np.asarray(x[b]).T.astype(bf)),
            "wqT": np.ascontiguousarray(np.asarray(wq)[sl].T.astype(bf)),
            "wkT": np.ascontiguousarray(np.asarray(wk)[sl].T.astype(bf)),
            "wvT": np.ascontiguousarray(np.asarray(wv)[sl].T.astype(bf)),
            "wpT": np.ascontiguousarray(np.asarray(wproj).T[sl].astype(bf)),
            "cosr": cosr,
            "sinr": sinr,
            "maskC": maskC,
            "ident": ident,
            "ones_in": np.ones((128, 1), dtype=bf),
        })
    return maps


def kernel(x, cos, sin, wq, wk, wv, wproj, _trace=False):
    nc = _get_nc()
    maps = _in_maps(x, cos, sin, wq, wk, wv, wproj)
    res = run_bass_kernel_spmd(nc, maps, core_ids=list(range(8)), trace=_trace)
    parts = [np.asarray(r["out"], dtype=np.float32) for r in res.results]
    outv = np.stack([parts[2 * b] + parts[2 * b + 1] for b in range(B)])
    if _trace:
        _CACHE["last_results"] = res
    return outv


# revision 13
# speedup vs baseline: 1.0849x; 1.0849x over previous
"""Causal self-attention (B=4, T=2048, C=768, H=6, D=128) on 8 trn2 NeuronCores.

Sharding: 24 (batch, head) units -> 8 cores, each core owns 1 batch x 3 heads.
Unshard: out[b] = partial[core 2b] + partial[core 2b+1]  (tensor-parallel sum).

v5 design notes (vs v4 two-phase):
  - INTERLEAVED schedule with a 9-tile stage-1 prefix: v4's strict two-phase
    ran 145us of gpsimd/DVE-bound stage-1 with the PE at 35%, then 137us of
    PE-bound attention with the vector engines idle. v5 front-loads 9 tiles
    of stage-1 (vector-bound, PE warms up on QKV), then spreads the 7
    remaining stage-1 tiles + transposes into attention chunks 0-2 (whose PE
    work they hide under); chunk 3 (the PE-fattest) runs pure.
  - Chunk qc's first score matmul reads QT columns for q-tiles 4qc..4qc+3,
    so those transposes must complete BEFORE the chunk starts (not
    just-in-time per key tile) — the extras tables below respect that.
  - stage-1 op fixes: rope swap-halves via strided 2x-mode DVE TT reading
    qk_sb directly (v4's materialized swap cost 2.9us/tile of gpsimd);
    cos/sin host-expanded to [T,768] with signs baked so every operand is
    contiguous; engine split tuned to DVE 4.4us / gpsimd 4.6us per tile.
  - proj of chunk qc-1 is inserted after head 1 (not head 0) of chunk qc so
    the chunk_end reciprocal/broadcast chain can finish without stalling
    the PE FIFO.
  - softmax denominator reciprocal via reciprocal_approx_fast (fp32, ~18
    bits); output in bf16 (halves out DMA); host sums TP pairs in fp32.
"""

import numpy as np
import ml_dtypes

import concourse.bacc as bacc
import concourse.bass as bass
import concourse.mybir as mybir
from concourse import tile
from concourse.bass_utils import run_bass_kernel_spmd

F32 = mybir.dt.float32
BF16 = mybir.dt.bfloat16
AF = mybir.ActivationFunctionType
ALU = mybir.AluOpType

B, T, C, H, D = 4, 2048, 768, 6, 128
HALF = D // 2
NH = 3            # heads per core
CT = C // 128     # 6 contraction tiles for projections
NT = T // 128     # 16 token tiles
QC = 512          # query-chunk width for attention
NQC = T // QC     # 4 chunks
SCALE = 1.0 / float(np.sqrt(D))
EPS = 1e-6
PREFIX = 9        # stage-1 tiles emitted before attention starts

_CACHE = {}


def _build_nc():
    nc = bacc.Bacc("TRN2")

    xT = nc.dram_tensor("xT", [C, T], BF16, kind="ExternalInput")
    wqT = nc.dram_tensor("wqT", [C, NH * D], BF16, kind="ExternalInput")
    wkT = nc.dram_tensor("wkT", [C, NH * D], BF16, kind="ExternalInput")
    wvT = nc.dram_tensor("wvT", [C, NH * D], BF16, kind="ExternalInput")
    wpT = nc.dram_tensor("wpT", [NH * D, C], BF16, kind="ExternalInput")
    cosr = nc.dram_tensor("cosr", [T, 2 * NH * D], BF16, kind="ExternalInput")
    sinr = nc.dram_tensor("sinr", [T, 2 * NH * D], BF16, kind="ExternalInput")
    maskC = nc.dram_tensor("maskC", [128, QC], BF16, kind="ExternalInput")
    ident = nc.dram_tensor("ident", [128, 128], BF16, kind="ExternalInput")
    ones_in = nc.dram_tensor("ones_in", [128, 1], BF16, kind="ExternalInput")
    out = nc.dram_tensor("out", [T, C], BF16, kind="ExternalOutput")

    with tile.TileContext(nc) as tc:
        with (
            tc.tile_pool(name="persist", bufs=1) as persist,
            tc.tile_pool(name="qkvbuf", bufs=1) as qkvbuf,
            tc.tile_pool(name="wbuf", bufs=1) as wbuf,
            tc.tile_pool(name="qkp", bufs=3) as qkpool,
            tc.tile_pool(name="rope", bufs=3) as rpool,
            tc.tile_pool(name="nrmp", bufs=6) as npool,
            tc.tile_pool(name="stat", bufs=4) as spool,
            tc.tile_pool(name="att", bufs=5) as apool,
            tc.tile_pool(name="acc", bufs=2) as accpool,
            tc.tile_pool(name="ybuf", bufs=2) as ypool,
            tc.tile_pool(name="obuf", bufs=3) as opool,
        ):
            QT = qkvbuf.tile([128, NH, T], BF16)       # [d, h, t]
            KT = qkvbuf.tile([128, NH, T], BF16)       # [d, h, t]
            V = qkvbuf.tile([128, NT, NH * D], BF16)   # [s%128, s//128, h*D+d]
            ones = persist.tile([128, 1], BF16)
            idn = persist.tile([128, 128], BF16)
            mask = persist.tile([128, QC], BF16)
            wp_sb = persist.tile([128, NH, C], BF16)   # [d, h, c]

            wq_sb = wbuf.tile([128, CT, NH * D], BF16)
            wk_sb = wbuf.tile([128, CT, NH * D], BF16)
            wv_sb = wbuf.tile([128, CT, NH * D], BF16)
            x_sb = wbuf.tile([128, CT, T], BF16)       # [c%128, c//128, t]

            wqT_r = wqT.rearrange("(ci p) o -> p ci o", p=128)
            nc.sync.dma_start(wq_sb[:], wqT_r[:])
            xT_r = xT.rearrange("(ci p) (g t) -> p ci g t", p=128, g=4)
            for g in range(4):
                nc.sync.dma_start(
                    x_sb[:].rearrange("p ci (g t) -> p ci g t", g=4)[:, :, g],
                    xT_r[:, :, g])
            nc.sync.dma_start(wk_sb[:], wkT.rearrange("(ci p) o -> p ci o", p=128))
            nc.sync.dma_start(wv_sb[:], wvT.rearrange("(ci p) o -> p ci o", p=128))

            # [c|c] per (m, h) and [s|-s] per (m, h), host-expanded
            cos_sb = wbuf.tile([128, NT, 2 * NH * D], BF16)
            sin_sb = wbuf.tile([128, NT, 2, NH, 2, HALF], BF16)
            nc.sync.dma_start(cos_sb[:], cosr.rearrange("(tt p) f -> p tt f", p=128))
            nc.sync.dma_start(
                sin_sb[:].rearrange("p tt m h two f -> p tt (m h two f)"),
                sinr.rearrange("(tt p) f -> p tt f", p=128))
            nc.sync.dma_start(idn[:], ident[:])
            nc.sync.dma_start(mask[:], maskC[:])
            nc.sync.dma_start(ones[:], ones_in[:])
            nc.sync.dma_start(wp_sb[:], wpT.rearrange("(h p) c -> p h c", p=128))

            out_r = out.rearrange("(tt p) c -> p tt c", p=128)

            # ---------------- stage 1 emitters ------------------------------
            nrm_of = {}
            s1_state = {}
            pools = {}

            def stage1_a(tt):
                # q in bank 0, k in bank 1, v in bank 2 of one psum tile
                qkv = pools["psq"].tile([128, 3, 512], F32, tag="qkv")
                for ci in range(CT):
                    st_, sp_ = (ci == 0), (ci == CT - 1)
                    lhs = x_sb[:, ci, tt * 128:(tt + 1) * 128]
                    nc.tensor.matmul(qkv[:, 0, 0:NH * D], lhs, wq_sb[:, ci, :],
                                     start=st_, stop=sp_, skip_group_check=True)
                    nc.tensor.matmul(qkv[:, 1, 0:NH * D], lhs, wk_sb[:, ci, :],
                                     start=st_, stop=sp_, skip_group_check=True)
                    nc.tensor.matmul(qkv[:, 2, 0:NH * D], lhs, wv_sb[:, ci, :],
                                     start=st_, stop=sp_, skip_group_check=True)

                # V: straight copy PSUM -> SBUF (bf16) in natural [t, o]
                nc.scalar.copy(V[:, tt, :], qkv[:, 2, 0:NH * D])

                # park q,k in SBUF bf16 (ACT reads PSUM)
                qk_sb = qkpool.tile([128, 2, NH, D], BF16, tag="qksb")
                nc.scalar.copy(qk_sb[:].rearrange("p m h f -> p m (h f)"),
                               qkv[:, 0:2, 0:NH * D])

                # rope: r = qk (.) [c|c]  +  swap(qk) (.) [s|-s]
                # all big TT on DVE (2x mode); w halves as strided 2x TT
                # reading qk_sb directly (no materialized swap).
                u = rpool.tile([128, 2, NH, D], BF16, tag="u")
                w = rpool.tile([128, 2, NH, D], BF16, tag="w")
                r = rpool.tile([128, 2, NH, D], BF16, tag="r")
                nc.vector.tensor_mul(u[:].rearrange("p m h f -> p (m h f)"),
                                     qk_sb[:].rearrange("p m h f -> p (m h f)"),
                                     cos_sb[:, tt])
                nc.vector.tensor_mul(w[:, :, :, 0:HALF], qk_sb[:, :, :, HALF:D],
                                     sin_sb[:, tt, :, :, 0, :])
                nc.vector.tensor_mul(w[:, :, :, HALF:D], qk_sb[:, :, :, 0:HALF],
                                     sin_sb[:, tt, :, :, 1, :])
                nc.vector.tensor_add(r[:], u[:], w[:])

                # sums per (token, head) via accumulating ACT copies (the
                # DVE reduce has no fast mode; ACT has slack)
                r6 = r[:].rearrange("p m h f -> p (m h) f")
                sums = spool.tile([128, 2 * NH], F32, tag="sums")
                scr = rpool.tile([128, 2, NH, D], BF16, tag="scr")
                for c in range(2 * NH):
                    nc.scalar.activation(
                        scr[:].rearrange("p m h f -> p (m h) f")[:, c], r6[:, c],
                        AF.Copy, accum_out=sums[:, c:c + 1])
                junk = rpool.tile([128, 2, NH, D], BF16, tag="junk")
                nc.gpsimd.tensor_mul(junk[:], r[:], r[:])
                sumsq = spool.tile([128, 2 * NH], F32, tag="sumsq")
                nc.vector.tensor_reduce(
                    sumsq[:], junk[:].rearrange("p m h f -> p (m h) f"),
                    axis=mybir.AxisListType.X, op=ALU.add)
                negmean = spool.tile([128, 2 * NH], F32, tag="negmean")
                nc.gpsimd.tensor_scalar_mul(negmean[:], sums[:], -1.0 / D)
                var = spool.tile([128, 2 * NH], F32, tag="var")
                nc.gpsimd.tensor_mul(var[:], negmean[:], sums[:])
                nc.gpsimd.tensor_add(var[:], sumsq[:], var[:])
                s1_state[tt] = (r, negmean, var)

            def stage1_b(tt):
                r, negmean, var = s1_state.pop(tt)
                stdv = spool.tile([128, 2 * NH], F32, tag="stdv")
                nc.scalar.activation(stdv[:], var[:], AF.Sqrt, scale=1.0 / (D - 1))
                nc.gpsimd.tensor_scalar_add(stdv[:], stdv[:], EPS)
                rstd = spool.tile([128, 2 * NH], F32, tag="rstd")
                nc.vector.reciprocal(rstd[:], stdv[:])

                # normalize: nrm = (r + negmean) * rstd  (q on DVE, k on gpsimd)
                nrm = npool.tile([128, 2, NH, D], BF16, tag="nrm")
                for mi in range(2):
                    for h in range(NH):
                        c = mi * NH + h
                        eng = nc.vector if mi == 0 else nc.gpsimd
                        eng.tensor_scalar(
                            nrm[:, mi, h], r[:, mi, h],
                            negmean[:, c:c + 1], rstd[:, c:c + 1],
                            op0=ALU.add, op1=ALU.mult)
                nrm_of[tt] = nrm

            def stage1_emit(tt):
                stage1_a(tt)
                stage1_b(tt)

            def emit_tp(tt):
                # SBUF->SBUF transpose on the DMA XBAR path: no PE/ACT/PSUM
                nrm = nrm_of.pop(tt)
                for mi in range(2):
                    dstT = QT if mi == 0 else KT
                    for h in range(NH):
                        nc.sync.dma_start_transpose(
                            dstT[:, h, tt * 128:(tt + 1) * 128], nrm[:, mi, h])

            # ---------------- attention chunk emitters -----------------------
            def emit_proj(qc, yTc):
                quanta = []
                for j in range(QC // 128):
                    def thunk(j=j, qc=qc, yTc=yTc):
                        tt = qc * (QC // 128) + j
                        op0 = pools["psS"].tile([128, C // 2], F32, tag="ps")
                        op1 = pools["psS"].tile([128, C // 2], F32, tag="ps")
                        for h in range(NH):
                            lhs = yTc[:, h, j * 128:(j + 1) * 128]
                            nc.tensor.matmul(op0[:], lhs, wp_sb[:, h, 0:C // 2],
                                             start=(h == 0), stop=(h == NH - 1))
                            nc.tensor.matmul(op1[:], lhs, wp_sb[:, h, C // 2:C],
                                             start=(h == 0), stop=(h == NH - 1))
                        ot = opool.tile([128, C], BF16, tag="ot")
                        nc.vector.tensor_copy(ot[:, 0:C // 2], op0[:])
                        nc.vector.tensor_copy(ot[:, C // 2:C], op1[:])
                        nc.sync.dma_start(out_r[:, tt, :], ot[:])
                    quanta.append(thunk)
                return quanta

            def attn_quanta(qc, pending_proj):
                Q0 = qc * QC
                n_st = (Q0 + QC) // 128
                state = {}
                ets = {}

                def start_chunk():
                    state["dps"] = pools["psD"].tile([128, QC], F32, tag="dps", name="dps")
                    state["yU"] = ypool.tile([128, NH, QC], BF16, tag="yU",
                                             name="yU")

                def loc0_of(st):
                    j = st - (n_st - 4)
                    return 128 * j if j > 0 else 0

                def emit_score(h, st):
                    loc0 = loc0_of(st)
                    sps = pools["psS"].tile([128, QC], F32, tag="ps")
                    nc.tensor.matmul(
                        sps[:, loc0:QC],
                        KT[:, h, st * 128:(st + 1) * 128],
                        QT[:, h, Q0 + loc0:Q0 + QC],
                        start=True, stop=True)
                    et = apool.tile([128, QC], BF16, tag="et")
                    nc.scalar.activation(et[:, loc0:QC], sps[:, loc0:QC],
                                         AF.Exp, scale=SCALE)
                    if st * 128 >= Q0:  # diagonal block: zero where s > q
                        nc.vector.tensor_mul(et[:, loc0:QC], et[:, loc0:QC],
                                             mask[:, 0:QC - loc0])
                    ets[(h, st)] = et

                def emit_av(h, st):
                    loc0 = loc0_of(st)
                    et = ets.pop((h, st))
                    nc.tensor.matmul(
                        state["yps"][:, loc0:QC],
                        V[:, st, h * D:(h + 1) * D],
                        et[:, loc0:QC],
                        start=(st == 0), stop=(st == n_st - 1),
                        skip_group_check=True)
                    nc.tensor.matmul(
                        state["dps"][32 * h:32 * h + 1, loc0:QC],
                        ones[:],
                        et[:, loc0:QC],
                        start=(st == 0), stop=(st == n_st - 1),
                        skip_group_check=True)

                quanta = [start_chunk]
                for h in range(NH):
                    def head_start(h=h):
                        state["yps"] = pools["psY"].tile([128, QC], F32, tag="yps",
                                                name="yps")
                        emit_score(h, 0)
                        if n_st > 1:
                            emit_score(h, 1)
                    quanta.append(head_start)
                    for st in range(n_st):
                        def block(h=h, st=st):
                            if st + 2 < n_st:
                                emit_score(h, st + 2)
                            emit_av(h, st)
                        quanta.append(block)
                    if h == 1 and pending_proj:
                        quanta.extend(pending_proj)

                    def head_end(h=h):
                        nc.vector.tensor_copy(state["yU"][:, h, :],
                                              state["yps"][:])
                    quanta.append(head_end)

                def chunk_end():
                    dps, yU = state["dps"], state["yU"]
                    rcf = accpool.tile([65, QC], F32, tag="rcf")
                    nc.vector.reciprocal_approx_fast(rcf[:], dps[0:65, :])
                    yTc = ypool.tile([128, NH, QC], BF16, tag="yT")
                    for h in range(NH):
                        # broadcast reads garbage from base partition != 0:
                        # stage each head's row at partition 0 first
                        rch = accpool.tile([1, QC], BF16, tag="rch")
                        nc.vector.tensor_copy(rch[:], rcf[32 * h:32 * h + 1, :])
                        rbc = accpool.tile([128, QC], BF16, tag="rbc")
                        nc.gpsimd.partition_broadcast(rbc[:], rch[:])
                        nc.vector.tensor_mul(yTc[:, h, :], yU[:, h, :], rbc[:])
                    state["yTc"] = yTc
                quanta.append(chunk_end)
                return quanta, state

            # ---------------- emission schedule ------------------------------
            # Prefix runs with double-buffered qkv psum (6 banks) + psT x2 —
            # the attention banks aren't needed yet. The scopes then swap.
            with (
                tc.tile_pool(name="psQKVa", bufs=2, space="PSUM") as psq_a,
            ):
                pools["psq"] = psq_a
                for tt in range(PREFIX):
                    stage1_emit(tt)
                    if tt >= 4:
                        emit_tp(tt - 4)   # tp 0..4

            def sa(tt):
                return lambda: stage1_a(tt)

            def sb(*tts):
                # paired: keeps the Sqrt ACT ops adjacent (one table reload
                # per pair instead of one per tile when interleaved with Exp)
                def thunk():
                    for tt in tts:
                        stage1_b(tt)
                return thunk

            def t(tt):
                return lambda: emit_tp(tt)

            extras = {
                0: [sa(9), t(5), sa(10), sb(9, 10), t(6), t(7)],
                1: [sa(11), t(8), sa(12), sb(11, 12), t(9), t(10), t(11)],
                2: [sa(13), sa(14), sb(13, 14), t(12), sa(15), sb(15), t(13),
                    t(14), t(15)],
                3: [],
            }

            with (
                tc.tile_pool(name="psQKVb", bufs=1, space="PSUM") as psq_b,
                tc.tile_pool(name="psS", bufs=2, space="PSUM") as psS_p,
                tc.tile_pool(name="psY", bufs=2, space="PSUM") as psY_p,
                tc.tile_pool(name="psD", bufs=1, space="PSUM") as psD_p,
            ):
                pools["psq"] = psq_b
                pools["psS"], pools["psY"], pools["psD"] = psS_p, psY_p, psD_p
                pending_proj = None
                for qc in range(NQC):
                    quanta, state = attn_quanta(qc, pending_proj)
                    ext = extras[qc]
                    stride = max(1, len(quanta) // (len(ext) + 1)) if ext else 0
                    ei = 0
                    for i, q in enumerate(quanta):
                        q()
                        if ext and ei < len(ext) and i % stride == stride - 1:
                            ext[ei]()
                            ei += 1
                    while ext and ei < len(ext):
                        ext[ei]()
                        ei += 1
                    pending_proj = emit_proj(qc, state["yTc"])
                for q in pending_proj:
                    q()

    nc.compile()
    return nc


def _get_nc():
    if "nc" not in _CACHE:
        _CACHE["nc"] = _build_nc()
    return _CACHE["nc"]


def _in_maps(x, cos, sin, wq, wk, wv, wproj):
    bf = ml_dtypes.bfloat16
    cos_np = np.asarray(cos, dtype=np.float32)
    sin_np = np.asarray(sin, dtype=np.float32)
    # cosr: [c|c] per (m, h); sinr: [s|-s] per (m, h); both [T, 768]
    cosr = np.ascontiguousarray(
        np.tile(np.concatenate([cos_np, cos_np], axis=1), (1, 2 * NH)).astype(bf))
    sinr = np.ascontiguousarray(
        np.tile(np.concatenate([sin_np, -sin_np], axis=1), (1, 2 * NH)).astype(bf))
    maskC = np.ascontiguousarray(
        (np.arange(QC)[None, :] >= np.arange(128)[:, None]).astype(bf))
    ident = np.eye(128, dtype=bf)
    maps = []
    for c in range(8):
        b = c // 2
        hs = (c % 2) * NH
        sl = slice(hs * D, (hs + NH) * D)
        maps.append({
            "xT": np.ascontiguousarray(np.asarray(x[b]).T.astype(bf)),
            "wqT": np.ascontiguousarray(np.asarray(wq)[sl].T.astype(bf)),
            "wkT": np.ascontiguousarray(np.asarray(wk)[sl].T.astype(bf)),
            "wvT": np.ascontiguousarray(np.asarray(wv)[sl].T.astype(bf)),
            "wpT": np.ascontiguousarray(np.asarray(wproj).T[sl].astype(bf)),
            "cosr": cosr,
            "sinr": sinr,
            "maskC": maskC,
            "ident": ident,
            "ones_in": np.ones((128, 1), dtype=bf),
        })
    return maps


def kernel(x, cos, sin, wq, wk, wv, wproj, _trace=False):
    nc = _get_nc()
    maps = _in_maps(x, cos, sin, wq, wk, wv, wproj)
    res = run_bass_kernel_spmd(nc, maps, core_ids=list(range(8)), trace=_trace)
    parts = [np.asarray(r["out"], dtype=np.float32) for r in res.results]
    outv = np.stack([parts[2 * b] + parts[2 * b + 1] for b in range(B)])
    if _trace:
        _CACHE["last_results"] = res
    return outv


# revision 15
# speedup vs baseline: 1.1895x; 1.0964x over previous
"""Causal self-attention (B=4, T=2048, C=768, H=6, D=128) on 8 trn2 NeuronCores.

Sharding: 24 (batch, head) units -> 8 cores, each core owns 1 batch x 3 heads.
Unshard: out[b] = partial[core 2b] + partial[core 2b+1]  (tensor-parallel sum).

v5 design notes (vs v4 two-phase):
  - INTERLEAVED schedule with a 9-tile stage-1 prefix: v4's strict two-phase
    ran 145us of gpsimd/DVE-bound stage-1 with the PE at 35%, then 137us of
    PE-bound attention with the vector engines idle. v5 front-loads 9 tiles
    of stage-1 (vector-bound, PE warms up on QKV), then spreads the 7
    remaining stage-1 tiles + transposes into attention chunks 0-2 (whose PE
    work they hide under); chunk 3 (the PE-fattest) runs pure.
  - Chunk qc's first score matmul reads QT columns for q-tiles 4qc..4qc+3,
    so those transposes must complete BEFORE the chunk starts (not
    just-in-time per key tile) — the extras tables below respect that.
  - stage-1 op fixes: rope swap-halves via strided 2x-mode DVE TT reading
    qk_sb directly (v4's materialized swap cost 2.9us/tile of gpsimd);
    cos/sin host-expanded to [T,768] with signs baked so every operand is
    contiguous; engine split tuned to DVE 4.4us / gpsimd 4.6us per tile.
  - proj of chunk qc-1 is inserted after head 1 (not head 0) of chunk qc so
    the chunk_end reciprocal/broadcast chain can finish without stalling
    the PE FIFO.
  - softmax denominator reciprocal via reciprocal_approx_fast (fp32, ~18
    bits); output in bf16 (halves out DMA); host sums TP pairs in fp32.
"""

import numpy as np
import ml_dtypes

import concourse.bacc as bacc
import concourse.bass as bass
import concourse.mybir as mybir
from concourse import tile
from concourse.bass_utils import run_bass_kernel_spmd

F32 = mybir.dt.float32
BF16 = mybir.dt.bfloat16
AF = mybir.ActivationFunctionType
ALU = mybir.AluOpType

B, T, C, H, D = 4, 2048, 768, 6, 128
HALF = D // 2
NH = 3            # heads per core
CT = C // 128     # 6 contraction tiles for projections
NT = T // 128     # 16 token tiles
QC = 512          # query-chunk width for attention
NQC = T // QC     # 4 chunks
SCALE = 1.0 / float(np.sqrt(D))
EPS = 1e-6
PREFIX = 9        # stage-1 tiles emitted before attention starts

_CACHE = {}


def _build_nc():
    nc = bacc.Bacc("TRN2")

    xT = nc.dram_tensor("xT", [C, T], BF16, kind="ExternalInput")
    wqT = nc.dram_tensor("wqT", [C, NH * D], BF16, kind="ExternalInput")
    wkT = nc.dram_tensor("wkT", [C, NH * D], BF16, kind="ExternalInput")
    wvT = nc.dram_tensor("wvT", [C, NH * D], BF16, kind="ExternalInput")
    wpT = nc.dram_tensor("wpT", [NH * D, C], BF16, kind="ExternalInput")
    cosr = nc.dram_tensor("cosr", [T, 2 * NH * D], BF16, kind="ExternalInput")
    sinr = nc.dram_tensor("sinr", [T, 2 * NH * D], BF16, kind="ExternalInput")
    maskC = nc.dram_tensor("maskC", [128, QC], BF16, kind="ExternalInput")
    ident = nc.dram_tensor("ident", [128, 128], BF16, kind="ExternalInput")
    ones_in = nc.dram_tensor("ones_in", [128, 1], BF16, kind="ExternalInput")
    out = nc.dram_tensor("out", [T, C], BF16, kind="ExternalOutput")

    with tile.TileContext(nc) as tc:
        with (
            tc.tile_pool(name="persist", bufs=1) as persist,
            tc.tile_pool(name="qkvbuf", bufs=1) as qkvbuf,
            tc.tile_pool(name="wbuf", bufs=1) as wbuf,
            tc.tile_pool(name="qkp", bufs=3) as qkpool,
            tc.tile_pool(name="rope", bufs=3) as rpool,
            tc.tile_pool(name="nrmp", bufs=6) as npool,
            tc.tile_pool(name="stat", bufs=4) as spool,
            tc.tile_pool(name="att", bufs=5) as apool,
            tc.tile_pool(name="acc", bufs=2) as accpool,
            tc.tile_pool(name="ybuf", bufs=2) as ypool,
            tc.tile_pool(name="obuf", bufs=3) as opool,
        ):
            QT = qkvbuf.tile([128, NH, T], BF16)       # [d, h, t]
            KT = qkvbuf.tile([128, NH, T], BF16)       # [d, h, t]
            V = qkvbuf.tile([128, NT, NH * D], BF16)   # [s%128, s//128, h*D+d]
            ones = persist.tile([128, 1], BF16)
            idn = persist.tile([128, 128], BF16)
            mask = persist.tile([128, QC], BF16)
            wp_sb = persist.tile([128, NH, C], BF16)   # [d, h, c]

            wq_sb = wbuf.tile([128, CT, NH * D], BF16)
            wk_sb = wbuf.tile([128, CT, NH * D], BF16)
            wv_sb = wbuf.tile([128, CT, NH * D], BF16)
            x_sb = wbuf.tile([128, CT, T], BF16)       # [c%128, c//128, t]

            wqT_r = wqT.rearrange("(ci p) o -> p ci o", p=128)
            nc.sync.dma_start(wq_sb[:], wqT_r[:])
            xT_r = xT.rearrange("(ci p) (g t) -> p ci g t", p=128, g=4)
            for g in range(4):
                nc.sync.dma_start(
                    x_sb[:].rearrange("p ci (g t) -> p ci g t", g=4)[:, :, g],
                    xT_r[:, :, g])
            nc.sync.dma_start(wk_sb[:], wkT.rearrange("(ci p) o -> p ci o", p=128))
            nc.sync.dma_start(wv_sb[:], wvT.rearrange("(ci p) o -> p ci o", p=128))

            # [c|c] per (m, h) and [s|-s] per (m, h), host-expanded
            cos_sb = wbuf.tile([128, NT, 2 * NH * D], BF16)
            sin_sb = wbuf.tile([128, NT, 2, NH, 2, HALF], BF16)
            nc.sync.dma_start(cos_sb[:], cosr.rearrange("(tt p) f -> p tt f", p=128))
            nc.sync.dma_start(
                sin_sb[:].rearrange("p tt m h two f -> p tt (m h two f)"),
                sinr.rearrange("(tt p) f -> p tt f", p=128))
            nc.sync.dma_start(idn[:], ident[:])
            nc.sync.dma_start(mask[:], maskC[:])
            nc.sync.dma_start(ones[:], ones_in[:])
            nc.sync.dma_start(wp_sb[:], wpT.rearrange("(h p) c -> p h c", p=128))

            out_r = out.rearrange("(tt p) c -> p tt c", p=128)

            # ---------------- stage 1 emitters ------------------------------
            nrm_of = {}
            s1_state = {}
            pools = {}

            def stage1_a(tt):
                # q in bank 0, k in bank 1, v in bank 2 of one psum tile
                qkv = pools["psq"].tile([128, 3, 512], F32, tag="qkv")
                for ci in range(CT):
                    st_, sp_ = (ci == 0), (ci == CT - 1)
                    lhs = x_sb[:, ci, tt * 128:(tt + 1) * 128]
                    nc.tensor.matmul(qkv[:, 0, 0:NH * D], lhs, wq_sb[:, ci, :],
                                     start=st_, stop=sp_, skip_group_check=True)
                    nc.tensor.matmul(qkv[:, 1, 0:NH * D], lhs, wk_sb[:, ci, :],
                                     start=st_, stop=sp_, skip_group_check=True)
                    nc.tensor.matmul(qkv[:, 2, 0:NH * D], lhs, wv_sb[:, ci, :],
                                     start=st_, stop=sp_, skip_group_check=True)

                # V: straight copy PSUM -> SBUF (bf16) in natural [t, o]
                nc.scalar.copy(V[:, tt, :], qkv[:, 2, 0:NH * D])

                # park q,k in SBUF bf16 (ACT reads PSUM)
                qk_sb = qkpool.tile([128, 2, NH, D], BF16, tag="qksb")
                nc.scalar.copy(qk_sb[:].rearrange("p m h f -> p m (h f)"),
                               qkv[:, 0:2, 0:NH * D])

                # rope: r = qk (.) [c|c]  +  swap(qk) (.) [s|-s]
                # all big TT on DVE (2x mode); w halves as strided 2x TT
                # reading qk_sb directly (no materialized swap).
                u = rpool.tile([128, 2, NH, D], BF16, tag="u")
                w = rpool.tile([128, 2, NH, D], BF16, tag="w")
                r = rpool.tile([128, 2, NH, D], BF16, tag="r")
                nc.vector.tensor_mul(u[:].rearrange("p m h f -> p (m h f)"),
                                     qk_sb[:].rearrange("p m h f -> p (m h f)"),
                                     cos_sb[:, tt])
                nc.vector.tensor_mul(w[:, :, :, 0:HALF], qk_sb[:, :, :, HALF:D],
                                     sin_sb[:, tt, :, :, 0, :])
                nc.vector.tensor_mul(w[:, :, :, HALF:D], qk_sb[:, :, :, 0:HALF],
                                     sin_sb[:, tt, :, :, 1, :])
                nc.vector.tensor_add(r[:], u[:], w[:])

                r6 = r[:].rearrange("p m h f -> p (m h) f")
                sums = spool.tile([128, 2 * NH], F32, tag="sums")
                nc.vector.tensor_reduce(sums[:], r6, axis=mybir.AxisListType.X,
                                        op=ALU.add)
                junk = rpool.tile([128, 2, NH, D], BF16, tag="junk")
                nc.gpsimd.tensor_mul(junk[:], r[:], r[:])
                sumsq = spool.tile([128, 2 * NH], F32, tag="sumsq")
                nc.vector.tensor_reduce(
                    sumsq[:], junk[:].rearrange("p m h f -> p (m h) f"),
                    axis=mybir.AxisListType.X, op=ALU.add)
                negmean = spool.tile([128, 2 * NH], F32, tag="negmean")
                nc.gpsimd.tensor_scalar_mul(negmean[:], sums[:], -1.0 / D)
                var = spool.tile([128, 2 * NH], F32, tag="var")
                nc.gpsimd.tensor_mul(var[:], negmean[:], sums[:])
                nc.gpsimd.tensor_add(var[:], sumsq[:], var[:])
                s1_state[tt] = (r, negmean, var)

            def stage1_b(tt):
                r, negmean, var = s1_state.pop(tt)
                stdv = spool.tile([128, 2 * NH], F32, tag="stdv")
                nc.scalar.activation(stdv[:], var[:], AF.Sqrt, scale=1.0 / (D - 1))
                nc.gpsimd.tensor_scalar_add(stdv[:], stdv[:], EPS)
                rstd = spool.tile([128, 2 * NH], F32, tag="rstd")
                nc.vector.reciprocal(rstd[:], stdv[:])

                # normalize: nrm = (r + negmean) * rstd  (q on DVE, k on gpsimd)
                nrm = npool.tile([128, 2, NH, D], BF16, tag="nrm")
                for mi in range(2):
                    for h in range(NH):
                        c = mi * NH + h
                        eng = nc.vector if mi == 0 else nc.gpsimd
                        eng.tensor_scalar(
                            nrm[:, mi, h], r[:, mi, h],
                            negmean[:, c:c + 1], rstd[:, c:c + 1],
                            op0=ALU.add, op1=ALU.mult)
                nrm_of[tt] = nrm

            def stage1_emit(tt):
                stage1_a(tt)
                stage1_b(tt)

            def emit_tp(tt):
                nrm = nrm_of.pop(tt)
                for mi in range(2):
                    dstT = QT if mi == 0 else KT
                    tps = pools["pst"].tile([128, NH * D], BF16, tag="tp")
                    for h in range(NH):
                        nc.tensor.transpose(
                            tps[:, h * D:(h + 1) * D], nrm[:, mi, h], idn[:])
                    dst = dstT[:, :, tt * 128:(tt + 1) * 128]
                    src = tps[:].rearrange("p (h t) -> p h t", h=NH)
                    nc.scalar.copy(dst, src)

            # ---------------- attention chunk emitters -----------------------
            def emit_proj(qc, yTc):
                quanta = []
                for j in range(QC // 128):
                    def thunk(j=j, qc=qc, yTc=yTc):
                        tt = qc * (QC // 128) + j
                        op0 = pools["psS"].tile([128, C // 2], F32, tag="ps")
                        op1 = pools["psS"].tile([128, C // 2], F32, tag="ps")
                        for h in range(NH):
                            lhs = yTc[:, h, j * 128:(j + 1) * 128]
                            nc.tensor.matmul(op0[:], lhs, wp_sb[:, h, 0:C // 2],
                                             start=(h == 0), stop=(h == NH - 1))
                            nc.tensor.matmul(op1[:], lhs, wp_sb[:, h, C // 2:C],
                                             start=(h == 0), stop=(h == NH - 1))
                        ot = opool.tile([128, C], BF16, tag="ot")
                        nc.vector.tensor_copy(ot[:, 0:C // 2], op0[:])
                        nc.vector.tensor_copy(ot[:, C // 2:C], op1[:])
                        nc.sync.dma_start(out_r[:, tt, :], ot[:])
                    quanta.append(thunk)
                return quanta

            def attn_quanta(qc, pending_proj):
                Q0 = qc * QC
                n_st = (Q0 + QC) // 128
                state = {}
                ets = {}

                def start_chunk():
                    state["dps"] = pools["psD"].tile([128, QC], F32, tag="dps", name="dps")
                    state["yU"] = ypool.tile([128, NH, QC], BF16, tag="yU",
                                             name="yU")

                def loc0_of(st):
                    j = st - (n_st - 4)
                    return 128 * j if j > 0 else 0

                def emit_score(h, st):
                    loc0 = loc0_of(st)
                    sps = pools["psS"].tile([128, QC], F32, tag="ps")
                    nc.tensor.matmul(
                        sps[:, loc0:QC],
                        KT[:, h, st * 128:(st + 1) * 128],
                        QT[:, h, Q0 + loc0:Q0 + QC],
                        start=True, stop=True)
                    et = apool.tile([128, QC], BF16, tag="et")
                    nc.scalar.activation(et[:, loc0:QC], sps[:, loc0:QC],
                                         AF.Exp, scale=SCALE)
                    if st * 128 >= Q0:  # diagonal block: zero where s > q
                        nc.vector.tensor_mul(et[:, loc0:QC], et[:, loc0:QC],
                                             mask[:, 0:QC - loc0])
                    ets[(h, st)] = et

                def emit_av(h, st):
                    loc0 = loc0_of(st)
                    et = ets.pop((h, st))
                    nc.tensor.matmul(
                        state["yps"][:, loc0:QC],
                        V[:, st, h * D:(h + 1) * D],
                        et[:, loc0:QC],
                        start=(st == 0), stop=(st == n_st - 1),
                        skip_group_check=True)
                    nc.tensor.matmul(
                        state["dps"][32 * h:32 * h + 1, loc0:QC],
                        ones[:],
                        et[:, loc0:QC],
                        start=(st == 0), stop=(st == n_st - 1),
                        skip_group_check=True)

                quanta = [start_chunk]
                for h in range(NH):
                    def head_start(h=h):
                        state["yps"] = pools["psY"].tile([128, QC], F32, tag="yps",
                                                name="yps")
                        emit_score(h, 0)
                        if n_st > 1:
                            emit_score(h, 1)
                    quanta.append(head_start)
                    for st in range(n_st):
                        def block(h=h, st=st):
                            if st + 2 < n_st:
                                emit_score(h, st + 2)
                            emit_av(h, st)
                        quanta.append(block)
                    if h == 1 and pending_proj:
                        quanta.extend(pending_proj)

                    def head_end(h=h):
                        nc.vector.tensor_copy(state["yU"][:, h, :],
                                              state["yps"][:])
                    quanta.append(head_end)

                def chunk_end():
                    dps, yU = state["dps"], state["yU"]
                    rcf = accpool.tile([65, QC], F32, tag="rcf")
                    nc.vector.reciprocal_approx_fast(rcf[:], dps[0:65, :])
                    yTc = ypool.tile([128, NH, QC], BF16, tag="yT")
                    for h in range(NH):
                        # broadcast reads garbage from base partition != 0:
                        # stage each head's row at partition 0 first
                        rch = accpool.tile([1, QC], BF16, tag="rch")
                        nc.vector.tensor_copy(rch[:], rcf[32 * h:32 * h + 1, :])
                        rbc = accpool.tile([128, QC], BF16, tag="rbc")
                        nc.gpsimd.partition_broadcast(rbc[:], rch[:])
                        nc.vector.tensor_mul(yTc[:, h, :], yU[:, h, :], rbc[:])
                    state["yTc"] = yTc
                quanta.append(chunk_end)
                return quanta, state

            # ---------------- emission schedule ------------------------------
            # Prefix runs with double-buffered qkv psum (6 banks) + psT x2 —
            # the attention banks aren't needed yet. The scopes then swap.
            with (
                tc.tile_pool(name="psQKVa", bufs=2, space="PSUM") as psq_a,
                tc.tile_pool(name="psTa", bufs=2, space="PSUM") as pst_a,
            ):
                pools["psq"], pools["pst"] = psq_a, pst_a
                for tt in range(PREFIX):
                    stage1_emit(tt)
                    if tt >= 4:
                        emit_tp(tt - 4)   # tp 0..4

            def sa(tt):
                return lambda: stage1_a(tt)

            def sb(*tts):
                # paired: keeps the Sqrt ACT ops adjacent (one table reload
                # per pair instead of one per tile when interleaved with Exp)
                def thunk():
                    for tt in tts:
                        stage1_b(tt)
                return thunk

            def t(tt):
                return lambda: emit_tp(tt)

            extras = {
                0: [sa(9), t(5), sa(10), sb(9, 10), t(6), t(7)],
                1: [sa(11), t(8), sa(12), sb(11, 12), t(9), t(10), t(11)],
                2: [sa(13), sa(14), sb(13, 14), t(12), sa(15), sb(15), t(13),
                    t(14), t(15)],
                3: [],
            }

            with (
                tc.tile_pool(name="psQKVb", bufs=1, space="PSUM") as psq_b,
                tc.tile_pool(name="psTb", bufs=1, space="PSUM") as pst_b,
                tc.tile_pool(name="psS", bufs=2, space="PSUM") as psS_p,
                tc.tile_pool(name="psY", bufs=1, space="PSUM") as psY_p,
                tc.tile_pool(name="psD", bufs=1, space="PSUM") as psD_p,
            ):
                pools["psq"], pools["pst"] = psq_b, pst_b
                pools["psS"], pools["psY"], pools["psD"] = psS_p, psY_p, psD_p
                pending_proj = None
                for qc in range(NQC):
                    quanta, state = attn_quanta(qc, pending_proj)
                    ext = extras[qc]
                    stride = max(1, len(quanta) // (len(ext) + 1)) if ext else 0
                    ei = 0
                    for i, q in enumerate(quanta):
                        q()
                        if ext and ei < len(ext) and i % stride == stride - 1:
                            ext[ei]()
                            ei += 1
                    while ext and ei < len(ext):
                        ext[ei]()
                        ei += 1
                    pending_proj = emit_proj(qc, state["yTc"])
                for q in pending_proj:
                    q()

    nc.compile()
    return nc


def _get_nc():
    if "nc" not in _CACHE:
        _CACHE["nc"] = _build_nc()
    return _CACHE["nc"]


def _in_maps(x, cos, sin, wq, wk, wv, wproj):
    bf = ml_dtypes.bfloat16
    cos_np = np.asarray(cos, dtype=np.float32)
    sin_np = np.asarray(sin, dtype=np.float32)
    # cosr: [c|c] per (m, h); sinr: [s|-s] per (m, h); both [T, 768]
    cosr = np.ascontiguousarray(
        np.tile(np.concatenate([cos_np, cos_np], axis=1), (1, 2 * NH)).astype(bf))
    sinr = np.ascontiguousarray(
        np.tile(np.concatenate([sin_np, -sin_np], axis=1), (1, 2 * NH)).astype(bf))
    maskC = np.ascontiguousarray(
        (np.arange(QC)[None, :] >= np.arange(128)[:, None]).astype(bf))
    ident = np.eye(128, dtype=bf)
    maps = []
    for c in range(8):
        b = c // 2
        hs = (c % 2) * NH
        sl = slice(hs * D, (hs + NH) * D)
        maps.append({
            "xT": np.ascontiguousarray(np.asarray(x[b]).T.astype(bf)),
            "wqT": np.ascontiguousarray(np.asarray(wq)[sl].T.astype(bf)),
            "wkT": np.ascontiguousarray(np.asarray(wk)[sl].T.astype(bf)),
            "wvT": np.ascontiguousarray(np.asarray(wv)[sl].T.astype(bf)),
            "wpT": np.ascontiguousarray(np.asarray(wproj).T[sl].astype(bf)),
            "cosr": cosr,
            "sinr": sinr,
            "maskC": maskC,
            "ident": ident,
            "ones_in": np.ones((128, 1), dtype=bf),
        })
    return maps


def kernel(x, cos, sin, wq, wk, wv, wproj, _trace=False):
    nc = _get_nc()
    maps = _in_maps(x, cos, sin, wq, wk, wv, wproj)
    res = run_bass_kernel_spmd(nc, maps, core_ids=list(range(8)), trace=_trace)
    parts = [np.asarray(r["out"], dtype=np.float32) for r in res.results]
    outv = np.stack([parts[2 * b] + parts[2 * b + 1] for b in range(B)])
    if _trace:
        _CACHE["last_results"] = res
    return outv
